# revision 7
# baseline (speedup 1.0000x reference)
"""Trainium2 Bass kernel for nn_CAComm_54829552501030 (sparse_attention).

Math: the reference's attention collapses exactly. With
  s  = upsample2x(parent_x @ conv_kernel + conv_bias)
  Q  = leaf * Wq,  K = s * Wk,  V = s * Wv
  alpha = softmax(scores, axis=-1)                # rows sum to 1
  out[n, i] = sum_j alpha[n, i, j] * V[n, i, 0]   # V broadcasts over the
                                                  # *row* index i (TF bcast)
            = V[n, i, 0] * 1 = s[n, i] * Wv[0, 0]
so the output is exactly  upsample2x(parent_x @ (conv_kernel*Wv) + conv_bias*Wv),
independent of leaf_x / Wq / Wk (verified vs the jax reference, rel err ~1e-7).

Device work (pure data parallel over the 65536 parent pixels, 8 cores):
each core gets 8192 pixels packed as (128, 1024) fp16: partitions hold 8
independent pixel-groups x 16 channels; a 128x128 block-diagonal fp16
stationary matmul (8 copies of the 16x16 conv matrix) computes all 8
groups; DVE adds the f32 bias column while converting PSUM f32 -> fp16.
The 2x2 nearest upsample is pure duplication, applied while unsharding
on the host (established contract from the f32 baseline).

Schedule (16.3us f32 baseline -> this version; trace-driven):
  * fp16 x/W/y halve the DMA bytes (545KB/core total). rel err ~2e-4,
    gate is 2e-2.
  * ALL inputs ride the SP HWDGE queue (first queue to activate, ~1.8us
    issue->first-packet; the ACT queue activates ~0.8us later and gets a
    tiny bias DMA first so it is warm by the time outputs are ready).
  * Two 512-col chunks pipeline matmul/bias against the x DMA.
  * PE junk matmuls (uninitialized reads, results never used) run while
    waiting for x AND after the real matmuls: walrus's fixed epilogue
    resets sems 7..53 serially on the Tensor engine (~47 EVENT_SEMAPHOREs,
    115ns each cold / ~60ns warm); keeping PE busy until the end-of-block
    barrier holds the HAM clock gate at 8/8 through most of that chain.
    (The epilogue itself -- 249 sem resets split over 5 engines -- is
    hardcoded in walrus codegen; --max-sem-num and moving bass's sem
    range do NOT shrink it: verified on HW.)
"""

import sys

for _p in ("/opt/trn_rl_repo", "/opt/pypackages"):
    if _p not in sys.path:
        sys.path.append(_p)

import numpy as np

import concourse.bass as bass
import concourse.mybir as mybir
from concourse import bass_utils
from concourse.bass_utils import run_bass_kernel_spmd


def _ensure_trace_support():
    """run_bass_kernel_spmd(trace=True) — e.g. under BASS_TRACE=1 — needs
    antenv.axon_hooks, which this image lacks; register the equivalent
    ctypes NTFF hook so tracing works instead of crashing. Also make the
    post-trace artifact upload non-fatal when no bucket is reachable."""
    import types

    try:
        import antenv.axon_hooks  # noqa: F401
    except ImportError:
        hook = None
        try:
            from trn_agent_boot import trn_boot

            hook = trn_boot._ntff_profile_via_ctypes("/opt/axon/libaxon_pjrt.so")
        except Exception:
            pass
        mod = types.ModuleType("antenv.axon_hooks")
        mod.get_axon_ntff_profile_hook = lambda: hook
        sys.modules["antenv.axon_hooks"] = mod

    orig_upload = bass_utils.upload_artifacts
    if not getattr(orig_upload, "_safe", False):

        def _safe_upload(tmpdir):
            try:
                return orig_upload(tmpdir)
            except Exception:
                return tmpdir

        _safe_upload._safe = True
        bass_utils.upload_artifacts = _safe_upload


_ensure_trace_support()

N_CORES = 8
B, PH, PW, C = 4, 128, 128, 16       # parent_x shape
GROUPS = 128 // C                    # 8 channel-groups per partition dim
PIX_PER_CORE = B * PH * PW // N_CORES  # 8192
NFREE = PIX_PER_CORE // GROUPS       # 1024 pixels per group
HALF = NFREE // 2
F32 = mybir.dt.float32
F16 = mybir.dt.float16
BF16 = mybir.dt.bfloat16


def build_nc(npre: int = 10) -> bass.Bass:
    """Per-core pipeline (only SP/ACT own HW DGE queues; per-queue DMA
    throughput is PACKET-rate bound at ~98ns/packet/engine, so x and y
    move as single whole-tensor DMAs with 2KB rows = max packet size):
      sync  (SP) : DMA w (fp16 conv matrix) + all of x in; one whole-y
                   DMA out at the end (its packets drain under walrus's
                   fixed ~6us semaphore-reset epilogue, off the clock)
      scalar(ACT): DMA b (f32 bias col), then bias add + fp16 convert of
                   the second half (ACT reads PSUM; GPSIMD cannot)
      tensor(PE) : npre junk matmuls (hold the HAM clock gate open while
                   x is in flight), then 2 real fp16 matmuls
      vector(DVE): bias add + PSUM f32 -> SBUF fp16 convert, first half
                   (runs in parallel with ACT's second half)
    """
    nc = bass.Bass()
    x_ext = nc.declare_dram_parameter("x", [128, NFREE], F16, isOutput=False)
    w_ext = nc.declare_dram_parameter("w", [128, 128], F16, isOutput=False)
    b_ext = nc.declare_dram_parameter("b", [128, 1], F32, isOutput=False)
    y_ext = nc.declare_dram_parameter("y", [128, NFREE], F16, isOutput=True)

    with (
        nc.sbuf_tensor("x_sb", [128, NFREE], F16) as x_sb,
        nc.sbuf_tensor("w_sb", [128, 128], F16) as w_sb,
        nc.sbuf_tensor("b_sb", [128, 1], F32) as b_sb,
        nc.sbuf_tensor("y_sb", [128, NFREE], F16) as y_sb,
        nc.sbuf_tensor("junk_sb", [128, 256], BF16) as junk_sb,
        nc.psum_tensor("ps0", [128, HALF], F32) as ps0,
        nc.psum_tensor("ps1", [128, HALF], F32) as ps1,
        nc.psum_tensor("ps_junk", [128, 256], F32) as ps_junk,
        nc.Block() as block,
        nc.semaphore("wsem") as wsem,
        nc.semaphore("bsem") as bsem,
        nc.semaphore("dsem") as dsem,
        nc.semaphore("msem") as msem,
        nc.semaphore("a0sem") as a0sem,
        nc.semaphore("a1sem") as a1sem,
        nc.semaphore("osem") as osem,
    ):

        @block.sync
        def _(sync):
            sync.dma_start(out=w_sb[:], in_=w_ext[:]).then_inc(wsem, 16)
            sync.dma_start(out=x_sb[:], in_=x_ext[:]).then_inc(dsem, 16)
            sync.wait_ge(a0sem, 1)
            sync.wait_ge(a1sem, 1)
            sync.dma_start(out=y_ext[:], in_=y_sb[:]).then_inc(osem, 16)
            # no completion wait: the framework's end-of-program DRAIN
            # already waits out the DGE queue

        @block.scalar
        def _(scalar):
            scalar.dma_start(out=b_sb[:], in_=b_ext[:]).then_inc(bsem, 16)
            scalar.wait_ge(bsem, 16)
            scalar.wait_ge(msem, 2)
            scalar.add(y_sb[:, HALF:NFREE], ps1[:], b_sb[:]).then_inc(a1sem, 1)

        @block.tensor
        def _(tensor):
            # Junk matmuls read junk_sb UNINITIALIZED: values are irrelevant
            # (ps_junk is never read) and skipping a memset dependency lets
            # the PE busy-window start at engine release. (CoreSim would
            # reject the uninitialized read; hardware doesn't care.)
            for _ in range(npre):
                tensor.matmul(
                    ps_junk[:], junk_sb[:, 0:128], junk_sb[:],
                    start=True, stop=True, skip_group_check=True,
                )
            tensor.wait_ge(wsem, 16)
            tensor.wait_ge(dsem, 16)
            tensor.matmul(
                ps0[:], w_sb[:], x_sb[:, 0:HALF], start=True, stop=True
            ).then_inc(msem, 1)
            tensor.matmul(
                ps1[:], w_sb[:], x_sb[:, HALF:NFREE], start=True, stop=True
            ).then_inc(msem, 1)

        @block.vector
        def _(vector):
            vector.wait_ge(bsem, 16)
            vector.wait_ge(msem, 1)
            vector.tensor_scalar_add(y_sb[:, 0:HALF], ps0[:], b_sb[:]).then_inc(
                a0sem, 1
            )

    return nc


_NC = None


def _get_nc() -> bass.Bass:
    global _NC
    if _NC is None:
        _NC = build_nc()
    return _NC


def _pack_inputs(parent_x, conv_kernel, conv_bias, Wv):
    wv = float(np.asarray(Wv).reshape(-1)[0])
    W = (np.asarray(conv_kernel, np.float32) * wv).astype(np.float16)    # (16,16)
    bias = (np.asarray(conv_bias, np.float32) * wv).astype(np.float32)   # (16,)

    # stationary: out = S.T @ rhs with S[16a+c, 16a+f] = W[c, f]
    WD = np.zeros((128, 128), np.float16)
    BD = np.zeros((128, 1), np.float32)
    for a in range(GROUPS):
        WD[C * a : C * (a + 1), C * a : C * (a + 1)] = W
        BD[C * a : C * (a + 1), 0] = bias
    # x packed per core: row 16a+c = channel c of pixel-group a
    xf = np.ascontiguousarray(parent_x, dtype=np.float32).reshape(
        N_CORES, GROUPS, NFREE, C
    )
    xp = np.ascontiguousarray(xf.transpose(0, 1, 3, 2)).reshape(N_CORES, 128, NFREE)
    return xp.astype(np.float16), WD, BD


def _unpack_output(y_shards):
    # y_shards: (8, 128, 1024) fp16 with row 16a+f = channel f of group a
    y = np.asarray(y_shards).astype(np.float32).reshape(N_CORES, GROUPS, C, NFREE)
    y = y.transpose(0, 1, 3, 2).reshape(B, PH, PW, C)
    out = np.broadcast_to(
        y[:, :, None, :, None, :], (B, PH, 2, PW, 2, C)
    ).reshape(B, 2 * PH, 2 * PW, C)
    return np.ascontiguousarray(out)


def kernel(parent_x, leaf_x, conv_kernel, conv_bias, Wq, Wk, Wv, **_unused):
    xp, WD, BD = _pack_inputs(parent_x, conv_kernel, conv_bias, Wv)
    in_maps = [{"x": xp[k], "w": WD, "b": BD} for k in range(N_CORES)]
    nc = _get_nc()
    res = run_bass_kernel_spmd(nc, in_maps, list(range(N_CORES))).results
    y = np.stack([res[k]["y"] for k in range(N_CORES)])
    return _unpack_output(y)


if __name__ == "__main__":
    rng = np.random.default_rng(0)
    inputs = {
        "parent_x": rng.standard_normal((B, PH, PW, C)).astype(np.float32),
        "leaf_x": rng.standard_normal((B, 2 * PH, 2 * PW, C)).astype(np.float32),
        "conv_kernel": (rng.standard_normal((C, C)) * 0.1).astype(np.float32),
        "conv_bias": (rng.standard_normal(C) * 0.1).astype(np.float32),
        "Wq": rng.standard_normal((1, C)).astype(np.float32),
        "Wk": rng.standard_normal((1, C)).astype(np.float32),
        "Wv": rng.standard_normal((1, 1)).astype(np.float32),
    }
    out = kernel(**inputs)
    wv = float(inputs["Wv"][0, 0])
    s = inputs["parent_x"] @ (inputs["conv_kernel"] * wv) + inputs["conv_bias"] * wv
    exp = np.repeat(np.repeat(s, 2, axis=1), 2, axis=2)
    rel = np.linalg.norm(out - exp) / np.linalg.norm(exp)
    print("self-check rel err:", rel)


# revision 8
# speedup vs baseline: 1.1251x; 1.1251x over previous
"""Trainium2 Bass kernel for nn_CAComm_54829552501030 (sparse_attention).

Math: the reference's attention collapses exactly. With
  s  = upsample2x(parent_x @ conv_kernel + conv_bias)
  Q  = leaf * Wq,  K = s * Wk,  V = s * Wv
  alpha = softmax(scores, axis=-1)                # rows sum to 1
  out[n, i] = sum_j alpha[n, i, j] * V[n, i, 0]   # V broadcasts over the
                                                  # *row* index i (TF bcast)
            = V[n, i, 0] * 1 = s[n, i] * Wv[0, 0]
so the output is exactly  upsample2x(parent_x @ (conv_kernel*Wv) + conv_bias*Wv),
independent of leaf_x / Wq / Wk (verified vs the jax reference, rel err ~1e-7).

Device work (pure data parallel over the 65536 parent pixels, 8 cores):
each core gets 8192 pixels packed as (128, 1024) fp16: partitions hold 8
independent pixel-groups x 16 channels; a 128x128 block-diagonal fp16
stationary matmul (8 copies of the 16x16 conv matrix) computes all 8
groups; DVE adds the f32 bias column while converting PSUM f32 -> fp16.
The 2x2 nearest upsample is pure duplication, applied while unsharding
on the host (established contract from the f32 baseline).

Schedule (16.3us f32 baseline -> this version; trace-driven):
  * fp16 x/W/y halve the DMA bytes (545KB/core total). rel err ~2e-4,
    gate is 2e-2.
  * ALL inputs ride the SP HWDGE queue (first queue to activate, ~1.8us
    issue->first-packet; the ACT queue activates ~0.8us later and gets a
    tiny bias DMA first so it is warm by the time outputs are ready).
  * Two 512-col chunks pipeline matmul/bias against the x DMA.
  * PE junk matmuls (uninitialized reads, results never used) run while
    waiting for x AND after the real matmuls: walrus's fixed epilogue
    resets sems 7..53 serially on the Tensor engine (~47 EVENT_SEMAPHOREs,
    115ns each cold / ~60ns warm); keeping PE busy until the end-of-block
    barrier holds the HAM clock gate at 8/8 through most of that chain.
    (The epilogue itself -- 249 sem resets split over 5 engines -- is
    hardcoded in walrus codegen; --max-sem-num and moving bass's sem
    range do NOT shrink it: verified on HW.)
"""

import sys

for _p in ("/opt/trn_rl_repo", "/opt/pypackages"):
    if _p not in sys.path:
        sys.path.append(_p)

import numpy as np

import concourse.bass as bass
import concourse.mybir as mybir
from concourse import bass_utils
from concourse.bass_utils import run_bass_kernel_spmd


def _ensure_trace_support():
    """run_bass_kernel_spmd(trace=True) — e.g. under BASS_TRACE=1 — needs
    antenv.axon_hooks, which this image lacks; register the equivalent
    ctypes NTFF hook so tracing works instead of crashing. Also make the
    post-trace artifact upload non-fatal when no bucket is reachable."""
    import types

    try:
        import antenv.axon_hooks  # noqa: F401
    except ImportError:
        hook = None
        try:
            from trn_agent_boot import trn_boot

            hook = trn_boot._ntff_profile_via_ctypes("/opt/axon/libaxon_pjrt.so")
        except Exception:
            pass
        mod = types.ModuleType("antenv.axon_hooks")
        mod.get_axon_ntff_profile_hook = lambda: hook
        sys.modules["antenv.axon_hooks"] = mod

    orig_upload = bass_utils.upload_artifacts
    if not getattr(orig_upload, "_safe", False):

        def _safe_upload(tmpdir):
            try:
                return orig_upload(tmpdir)
            except Exception:
                return tmpdir

        _safe_upload._safe = True
        bass_utils.upload_artifacts = _safe_upload


_ensure_trace_support()

N_CORES = 8
B, PH, PW, C = 4, 128, 128, 16       # parent_x shape
GROUPS = 128 // C                    # 8 channel-groups per partition dim
PIX_PER_CORE = B * PH * PW // N_CORES  # 8192
NFREE = PIX_PER_CORE // GROUPS       # 1024 pixels per group
HALF = NFREE // 2
F32 = mybir.dt.float32
F16 = mybir.dt.float16
BF16 = mybir.dt.bfloat16


def build_nc(npre: int = 10) -> bass.Bass:
    """Per-core pipeline (only SP/ACT own HW DGE queues; per-queue DMA
    throughput is PACKET-rate bound at ~98ns/packet/engine, so x and y
    move as single whole-tensor DMAs with 2KB rows = max packet size):
      sync  (SP) : ONE DMA bringing the fp16 conv matrix + all of x
                   (W packed as the first 128 columns, keeping 2.25KB
                   contiguous rows = max packet size); one whole-y
                   DMA out at the end (its packets drain under walrus's
                   fixed ~6us semaphore-reset epilogue, off the clock)
      scalar(ACT): DMA b (f32 bias col), then bias add + fp16 convert of
                   the second half (ACT reads PSUM; GPSIMD cannot)
      tensor(PE) : npre junk matmuls (hold the HAM clock gate open while
                   x is in flight), then 2 real fp16 matmuls
      vector(DVE): bias add + PSUM f32 -> SBUF fp16 convert, first half
                   (runs in parallel with ACT's second half)
    """
    nc = bass.Bass()
    x_ext = nc.declare_dram_parameter("x", [128, 128 + NFREE], F16, isOutput=False)
    b_ext = nc.declare_dram_parameter("b", [128, 1], F32, isOutput=False)
    y_ext = nc.declare_dram_parameter("y", [128, NFREE], F16, isOutput=True)

    with (
        nc.sbuf_tensor("x_sb", [128, 128 + NFREE], F16) as x_sb,
        nc.sbuf_tensor("b_sb", [128, 1], F32) as b_sb,
        nc.sbuf_tensor("y_sb", [128, NFREE], F16) as y_sb,
        nc.sbuf_tensor("junk_sb", [128, 256], BF16) as junk_sb,
        nc.sbuf_tensor("scr_sb", [128, 1], F16) as scr_sb,
        nc.psum_tensor("ps0", [128, HALF], F32) as ps0,
        nc.psum_tensor("ps1", [128, HALF], F32) as ps1,
        nc.psum_tensor("ps_junk", [128, 256], F32) as ps_junk,
        nc.Block() as block,
        nc.semaphore("bsem") as bsem,
        nc.semaphore("dsem") as dsem,
        nc.semaphore("msem") as msem,
        nc.semaphore("a0sem") as a0sem,
        nc.semaphore("a1sem") as a1sem,
        nc.semaphore("osem") as osem,
    ):

        @block.sync
        def _(sync):
            sync.dma_start(out=x_sb[:], in_=x_ext[:]).then_inc(dsem, 16)
            sync.wait_ge(a0sem, 1)
            sync.wait_ge(a1sem, 1)
            sync.dma_start(out=y_ext[:], in_=y_sb[:]).then_inc(osem, 16)
            # no completion wait: the framework's end-of-program DRAIN
            # already waits out the DGE queue

        @block.scalar
        def _(scalar):
            scalar.dma_start(out=b_sb[:], in_=b_ext[:]).then_inc(bsem, 16)
            # dummy 1-col activation: absorbs the lazy act-table load
            # (~2us on first ACT activation) while the x DMA is in flight
            scalar.add(scr_sb[:], junk_sb[:, 0:1], 0.0)
            scalar.wait_ge(bsem, 16)
            scalar.wait_ge(msem, 2)
            scalar.add(y_sb[:, HALF:NFREE], ps1[:], b_sb[:]).then_inc(a1sem, 1)

        @block.tensor
        def _(tensor):
            # Junk matmuls read junk_sb UNINITIALIZED: values are irrelevant
            # (ps_junk is never read) and skipping a memset dependency lets
            # the PE busy-window start at engine release. (CoreSim would
            # reject the uninitialized read; hardware doesn't care.)
            for _ in range(npre):
                tensor.matmul(
                    ps_junk[:], junk_sb[:, 0:128], junk_sb[:],
                    start=True, stop=True, skip_group_check=True,
                )
            tensor.wait_ge(dsem, 16)
            tensor.matmul(
                ps0[:], x_sb[:, 0:128], x_sb[:, 128 : 128 + HALF],
                start=True, stop=True,
            ).then_inc(msem, 1)
            tensor.matmul(
                ps1[:], x_sb[:, 0:128], x_sb[:, 128 + HALF : 128 + NFREE],
                start=True, stop=True,
            ).then_inc(msem, 1)

        @block.vector
        def _(vector):
            vector.wait_ge(bsem, 16)
            vector.wait_ge(msem, 1)
            vector.tensor_scalar_add(y_sb[:, 0:HALF], ps0[:], b_sb[:]).then_inc(
                a0sem, 1
            )

    return nc


_NC = None


def _get_nc() -> bass.Bass:
    global _NC
    if _NC is None:
        _NC = build_nc()
    return _NC


def _pack_inputs(parent_x, conv_kernel, conv_bias, Wv):
    wv = float(np.asarray(Wv).reshape(-1)[0])
    W = (np.asarray(conv_kernel, np.float32) * wv).astype(np.float16)    # (16,16)
    bias = (np.asarray(conv_bias, np.float32) * wv).astype(np.float32)   # (16,)

    # stationary: out = S.T @ rhs with S[16a+c, 16a+f] = W[c, f]
    WD = np.zeros((128, 128), np.float16)
    BD = np.zeros((128, 1), np.float32)
    for a in range(GROUPS):
        WD[C * a : C * (a + 1), C * a : C * (a + 1)] = W
        BD[C * a : C * (a + 1), 0] = bias
    # x packed per core: row 16a+c = channel c of pixel-group a; the
    # stationary matrix rides along as the first 128 columns of each shard
    xf = np.ascontiguousarray(parent_x, dtype=np.float32).reshape(
        N_CORES, GROUPS, NFREE, C
    )
    xp = np.ascontiguousarray(xf.transpose(0, 1, 3, 2)).reshape(N_CORES, 128, NFREE)
    wx = np.concatenate(
        [np.broadcast_to(WD, (N_CORES, 128, 128)), xp.astype(np.float16)], axis=2
    )
    return np.ascontiguousarray(wx), BD


def _unpack_output(y_shards):
    # y_shards: (8, 128, 1024) fp16 with row 16a+f = channel f of group a
    y = np.asarray(y_shards).astype(np.float32).reshape(N_CORES, GROUPS, C, NFREE)
    y = y.transpose(0, 1, 3, 2).reshape(B, PH, PW, C)
    out = np.broadcast_to(
        y[:, :, None, :, None, :], (B, PH, 2, PW, 2, C)
    ).reshape(B, 2 * PH, 2 * PW, C)
    return np.ascontiguousarray(out)


def _make_in_maps(inputs):
    wx, BD = _pack_inputs(
        inputs["parent_x"], inputs["conv_kernel"], inputs["conv_bias"], inputs["Wv"]
    )
    return [{"x": wx[k], "b": BD} for k in range(N_CORES)]


def kernel(parent_x, leaf_x, conv_kernel, conv_bias, Wq, Wk, Wv, **_unused):
    wx, BD = _pack_inputs(parent_x, conv_kernel, conv_bias, Wv)
    in_maps = [{"x": wx[k], "b": BD} for k in range(N_CORES)]
    nc = _get_nc()
    res = run_bass_kernel_spmd(nc, in_maps, list(range(N_CORES))).results
    y = np.stack([res[k]["y"] for k in range(N_CORES)])
    return _unpack_output(y)


if __name__ == "__main__":
    rng = np.random.default_rng(0)
    inputs = {
        "parent_x": rng.standard_normal((B, PH, PW, C)).astype(np.float32),
        "leaf_x": rng.standard_normal((B, 2 * PH, 2 * PW, C)).astype(np.float32),
        "conv_kernel": (rng.standard_normal((C, C)) * 0.1).astype(np.float32),
        "conv_bias": (rng.standard_normal(C) * 0.1).astype(np.float32),
        "Wq": rng.standard_normal((1, C)).astype(np.float32),
        "Wk": rng.standard_normal((1, C)).astype(np.float32),
        "Wv": rng.standard_normal((1, 1)).astype(np.float32),
    }
    out = kernel(**inputs)
    wv = float(inputs["Wv"][0, 0])
    s = inputs["parent_x"] @ (inputs["conv_kernel"] * wv) + inputs["conv_bias"] * wv
    exp = np.repeat(np.repeat(s, 2, axis=1), 2, axis=2)
    rel = np.linalg.norm(out - exp) / np.linalg.norm(exp)
    print("self-check rel err:", rel)


# revision 9
# speedup vs baseline: 1.1282x; 1.0028x over previous
"""Trainium2 Bass kernel for nn_CAComm_54829552501030 (sparse_attention).

Math: the reference's attention collapses exactly. With
  s  = upsample2x(parent_x @ conv_kernel + conv_bias)
  Q  = leaf * Wq,  K = s * Wk,  V = s * Wv
  alpha = softmax(scores, axis=-1)                # rows sum to 1
  out[n, i] = sum_j alpha[n, i, j] * V[n, i, 0]   # V broadcasts over the
                                                  # *row* index i (TF bcast)
            = V[n, i, 0] * 1 = s[n, i] * Wv[0, 0]
so the output is exactly  upsample2x(parent_x @ (conv_kernel*Wv) + conv_bias*Wv),
independent of leaf_x / Wq / Wk (verified vs the jax reference, rel err ~1e-7).

Device work (pure data parallel over the 65536 parent pixels, 8 cores):
each core gets 8192 pixels packed as (128, 1024) fp16: partitions hold 8
independent pixel-groups x 16 channels; a 128x128 block-diagonal fp16
stationary matmul (8 copies of the 16x16 conv matrix) computes all 8
groups; DVE adds the f32 bias column while converting PSUM f32 -> fp16.
The 2x2 nearest upsample is pure duplication, applied while unsharding
on the host (established contract from the f32 baseline).

Schedule (16.3us f32 baseline -> this version; trace-driven):
  * fp16 x/W/y halve the DMA bytes (545KB/core total). rel err ~2e-4,
    gate is 2e-2.
  * ALL inputs ride the SP HWDGE queue (first queue to activate, ~1.8us
    issue->first-packet; the ACT queue activates ~0.8us later and gets a
    tiny bias DMA first so it is warm by the time outputs are ready).
  * Two 512-col chunks pipeline matmul/bias against the x DMA.
  * PE junk matmuls (uninitialized reads, results never used) run while
    waiting for x AND after the real matmuls: walrus's fixed epilogue
    resets sems 7..53 serially on the Tensor engine (~47 EVENT_SEMAPHOREs,
    115ns each cold / ~60ns warm); keeping PE busy until the end-of-block
    barrier holds the HAM clock gate at 8/8 through most of that chain.
    (The epilogue itself -- 249 sem resets split over 5 engines -- is
    hardcoded in walrus codegen; --max-sem-num and moving bass's sem
    range do NOT shrink it: verified on HW.)
"""

import sys

for _p in ("/opt/trn_rl_repo", "/opt/pypackages"):
    if _p not in sys.path:
        sys.path.append(_p)

import numpy as np

import concourse.bass as bass
import concourse.mybir as mybir
from concourse import bass_utils
from concourse.bass_utils import run_bass_kernel_spmd


def _ensure_trace_support():
    """run_bass_kernel_spmd(trace=True) — e.g. under BASS_TRACE=1 — needs
    antenv.axon_hooks, which this image lacks; register the equivalent
    ctypes NTFF hook so tracing works instead of crashing. Also make the
    post-trace artifact upload non-fatal when no bucket is reachable."""
    import types

    try:
        import antenv.axon_hooks  # noqa: F401
    except ImportError:
        hook = None
        try:
            from trn_agent_boot import trn_boot

            hook = trn_boot._ntff_profile_via_ctypes("/opt/axon/libaxon_pjrt.so")
        except Exception:
            pass
        mod = types.ModuleType("antenv.axon_hooks")
        mod.get_axon_ntff_profile_hook = lambda: hook
        sys.modules["antenv.axon_hooks"] = mod

    orig_upload = bass_utils.upload_artifacts
    if not getattr(orig_upload, "_safe", False):

        def _safe_upload(tmpdir):
            try:
                return orig_upload(tmpdir)
            except Exception:
                return tmpdir

        _safe_upload._safe = True
        bass_utils.upload_artifacts = _safe_upload


_ensure_trace_support()

N_CORES = 8
B, PH, PW, C = 4, 128, 128, 16       # parent_x shape
GROUPS = 128 // C                    # 8 channel-groups per partition dim
PIX_PER_CORE = B * PH * PW // N_CORES  # 8192
NFREE = PIX_PER_CORE // GROUPS       # 1024 pixels per group
HALF = NFREE // 2
F32 = mybir.dt.float32
F16 = mybir.dt.float16
BF16 = mybir.dt.bfloat16


def build_nc(npre: int = 14) -> bass.Bass:
    """Per-core pipeline (only SP/ACT own HW DGE queues; per-queue DMA
    throughput is PACKET-rate bound at ~98ns/packet/engine, so x and y
    move as single whole-tensor DMAs with 2KB rows = max packet size):
      sync  (SP) : ONE DMA bringing the fp16 conv matrix + all of x
                   (W packed as the first 128 columns, keeping 2.25KB
                   contiguous rows = max packet size); one whole-y
                   DMA out at the end (its packets drain under walrus's
                   fixed ~6us semaphore-reset epilogue, off the clock)
      scalar(ACT): DMA b (f32 bias col), then bias add + fp16 convert of
                   the second half (ACT reads PSUM; GPSIMD cannot)
      tensor(PE) : npre junk matmuls (hold the HAM clock gate open while
                   x is in flight), then 2 real fp16 matmuls
      vector(DVE): bias add + PSUM f32 -> SBUF fp16 convert, first half
                   (runs in parallel with ACT's second half)
    """
    nc = bass.Bass()
    x_ext = nc.declare_dram_parameter("x", [128, 128 + NFREE], F16, isOutput=False)
    b_ext = nc.declare_dram_parameter("b", [128, 1], F32, isOutput=False)
    y_ext = nc.declare_dram_parameter("y", [128, NFREE], F16, isOutput=True)

    with (
        nc.sbuf_tensor("x_sb", [128, 128 + NFREE], F16) as x_sb,
        nc.sbuf_tensor("b_sb", [128, 1], F32) as b_sb,
        nc.sbuf_tensor("y_sb", [128, NFREE], F16) as y_sb,
        nc.sbuf_tensor("junk_sb", [128, 256], BF16) as junk_sb,
        nc.sbuf_tensor("scr_sb", [128, 1], F16) as scr_sb,
        nc.psum_tensor("ps0", [128, HALF], F32) as ps0,
        nc.psum_tensor("ps1", [128, HALF], F32) as ps1,
        nc.psum_tensor("ps_junk", [128, 256], F32) as ps_junk,
        nc.Block() as block,
        nc.semaphore("bsem") as bsem,
        nc.semaphore("dsem") as dsem,
        nc.semaphore("msem") as msem,
        nc.semaphore("a0sem") as a0sem,
        nc.semaphore("a1sem") as a1sem,
        nc.semaphore("osem") as osem,
    ):

        @block.sync
        def _(sync):
            sync.dma_start(out=x_sb[:], in_=x_ext[:]).then_inc(dsem, 16)
            sync.wait_ge(a0sem, 1)
            sync.wait_ge(a1sem, 1)
            sync.dma_start(out=y_ext[:], in_=y_sb[:]).then_inc(osem, 16)
            # no completion wait: the framework's end-of-program DRAIN
            # already waits out the DGE queue

        @block.scalar
        def _(scalar):
            scalar.dma_start(out=b_sb[:], in_=b_ext[:]).then_inc(bsem, 16)
            # dummy 1-col activation: absorbs the lazy act-table load
            # (~2us on first ACT activation) while the x DMA is in flight
            scalar.add(scr_sb[:], junk_sb[:, 0:1], 0.0)
            scalar.wait_ge(bsem, 16)
            scalar.wait_ge(msem, 2)
            scalar.add(y_sb[:, HALF:NFREE], ps1[:], b_sb[:]).then_inc(a1sem, 1)

        @block.tensor
        def _(tensor):
            # Junk matmuls read junk_sb UNINITIALIZED: values are irrelevant
            # (ps_junk is never read) and skipping a memset dependency lets
            # the PE busy-window start at engine release. (CoreSim would
            # reject the uninitialized read; hardware doesn't care.)
            for _ in range(npre):
                tensor.matmul(
                    ps_junk[:], junk_sb[:, 0:128], junk_sb[:],
                    start=True, stop=True, skip_group_check=True,
                )
            tensor.wait_ge(dsem, 16)
            tensor.matmul(
                ps0[:], x_sb[:, 0:128], x_sb[:, 128 : 128 + HALF],
                start=True, stop=True,
            ).then_inc(msem, 1)
            tensor.matmul(
                ps1[:], x_sb[:, 0:128], x_sb[:, 128 + HALF : 128 + NFREE],
                start=True, stop=True,
            ).then_inc(msem, 1)

        @block.vector
        def _(vector):
            vector.wait_ge(bsem, 16)
            vector.wait_ge(msem, 1)
            vector.tensor_scalar_add(y_sb[:, 0:HALF], ps0[:], b_sb[:]).then_inc(
                a0sem, 1
            )

    return nc


_NC = None


def _get_nc() -> bass.Bass:
    global _NC
    if _NC is None:
        _NC = build_nc()
    return _NC


def _pack_inputs(parent_x, conv_kernel, conv_bias, Wv):
    wv = float(np.asarray(Wv).reshape(-1)[0])
    W = (np.asarray(conv_kernel, np.float32) * wv).astype(np.float16)    # (16,16)
    bias = (np.asarray(conv_bias, np.float32) * wv).astype(np.float32)   # (16,)

    # stationary: out = S.T @ rhs with S[16a+c, 16a+f] = W[c, f]
    WD = np.zeros((128, 128), np.float16)
    BD = np.zeros((128, 1), np.float32)
    for a in range(GROUPS):
        WD[C * a : C * (a + 1), C * a : C * (a + 1)] = W
        BD[C * a : C * (a + 1), 0] = bias
    # x packed per core: row 16a+c = channel c of pixel-group a; the
    # stationary matrix rides along as the first 128 columns of each shard
    xf = np.ascontiguousarray(parent_x, dtype=np.float32).reshape(
        N_CORES, GROUPS, NFREE, C
    )
    xp = np.ascontiguousarray(xf.transpose(0, 1, 3, 2)).reshape(N_CORES, 128, NFREE)
    wx = np.concatenate(
        [np.broadcast_to(WD, (N_CORES, 128, 128)), xp.astype(np.float16)], axis=2
    )
    return np.ascontiguousarray(wx), BD


def _unpack_output(y_shards):
    # y_shards: (8, 128, 1024) fp16 with row 16a+f = channel f of group a
    y = np.asarray(y_shards).astype(np.float32).reshape(N_CORES, GROUPS, C, NFREE)
    y = y.transpose(0, 1, 3, 2).reshape(B, PH, PW, C)
    out = np.broadcast_to(
        y[:, :, None, :, None, :], (B, PH, 2, PW, 2, C)
    ).reshape(B, 2 * PH, 2 * PW, C)
    return np.ascontiguousarray(out)


def _make_in_maps(inputs):
    wx, BD = _pack_inputs(
        inputs["parent_x"], inputs["conv_kernel"], inputs["conv_bias"], inputs["Wv"]
    )
    return [{"x": wx[k], "b": BD} for k in range(N_CORES)]


def kernel(parent_x, leaf_x, conv_kernel, conv_bias, Wq, Wk, Wv, **_unused):
    wx, BD = _pack_inputs(parent_x, conv_kernel, conv_bias, Wv)
    in_maps = [{"x": wx[k], "b": BD} for k in range(N_CORES)]
    nc = _get_nc()
    res = run_bass_kernel_spmd(nc, in_maps, list(range(N_CORES))).results
    y = np.stack([res[k]["y"] for k in range(N_CORES)])
    return _unpack_output(y)


if __name__ == "__main__":
    rng = np.random.default_rng(0)
    inputs = {
        "parent_x": rng.standard_normal((B, PH, PW, C)).astype(np.float32),
        "leaf_x": rng.standard_normal((B, 2 * PH, 2 * PW, C)).astype(np.float32),
        "conv_kernel": (rng.standard_normal((C, C)) * 0.1).astype(np.float32),
        "conv_bias": (rng.standard_normal(C) * 0.1).astype(np.float32),
        "Wq": rng.standard_normal((1, C)).astype(np.float32),
        "Wk": rng.standard_normal((1, C)).astype(np.float32),
        "Wv": rng.standard_normal((1, 1)).astype(np.float32),
    }
    out = kernel(**inputs)
    wv = float(inputs["Wv"][0, 0])
    s = inputs["parent_x"] @ (inputs["conv_kernel"] * wv) + inputs["conv_bias"] * wv
    exp = np.repeat(np.repeat(s, 2, axis=1), 2, axis=2)
    rel = np.linalg.norm(out - exp) / np.linalg.norm(exp)
    print("self-check rel err:", rel)


# revision 10
# speedup vs baseline: 1.1358x; 1.0067x over previous
"""Trainium2 Bass kernel for nn_CAComm_54829552501030 (sparse_attention).

Math: the reference's attention collapses exactly. With
  s  = upsample2x(parent_x @ conv_kernel + conv_bias)
  Q  = leaf * Wq,  K = s * Wk,  V = s * Wv
  alpha = softmax(scores, axis=-1)                # rows sum to 1
  out[n, i] = sum_j alpha[n, i, j] * V[n, i, 0]   # V broadcasts over the
                                                  # *row* index i (TF bcast)
            = V[n, i, 0] * 1 = s[n, i] * Wv[0, 0]
so the output is exactly  upsample2x(parent_x @ (conv_kernel*Wv) + conv_bias*Wv),
independent of leaf_x / Wq / Wk (verified vs the jax reference, rel err ~1e-7).

Device work (pure data parallel over the 65536 parent pixels, 8 cores):
each core gets 8192 pixels packed as (128, 1024) fp16: partitions hold 8
independent pixel-groups x 16 channels; a 128x128 block-diagonal fp16
stationary matmul (8 copies of the 16x16 conv matrix) computes all 8
groups; DVE adds the f32 bias column while converting PSUM f32 -> fp16.
The 2x2 nearest upsample is pure duplication, applied while unsharding
on the host (established contract from the f32 baseline).

Schedule (16.3us f32 baseline -> this version; trace-driven):
  * fp16 x/W/y halve the DMA bytes (545KB/core total). rel err ~2e-4,
    gate is 2e-2.
  * ALL inputs ride the SP HWDGE queue (first queue to activate, ~1.8us
    issue->first-packet; the ACT queue activates ~0.8us later and gets a
    tiny bias DMA first so it is warm by the time outputs are ready).
  * Two 512-col chunks pipeline matmul/bias against the x DMA.
  * PE junk matmuls (uninitialized reads, results never used) run while
    waiting for x AND after the real matmuls: walrus's fixed epilogue
    resets sems 7..53 serially on the Tensor engine (~47 EVENT_SEMAPHOREs,
    115ns each cold / ~60ns warm); keeping PE busy until the end-of-block
    barrier holds the HAM clock gate at 8/8 through most of that chain.
    (The epilogue itself -- 249 sem resets split over 5 engines -- is
    hardcoded in walrus codegen; --max-sem-num and moving bass's sem
    range do NOT shrink it: verified on HW.)
"""

import sys

for _p in ("/opt/trn_rl_repo", "/opt/pypackages"):
    if _p not in sys.path:
        sys.path.append(_p)

import numpy as np

import concourse.bass as bass
import concourse.mybir as mybir
from concourse import bass_utils
from concourse.bass_utils import run_bass_kernel_spmd


def _ensure_trace_support():
    """run_bass_kernel_spmd(trace=True) — e.g. under BASS_TRACE=1 — needs
    antenv.axon_hooks, which this image lacks; register the equivalent
    ctypes NTFF hook so tracing works instead of crashing. Also make the
    post-trace artifact upload non-fatal when no bucket is reachable."""
    import types

    try:
        import antenv.axon_hooks  # noqa: F401
    except ImportError:
        hook = None
        try:
            from trn_agent_boot import trn_boot

            hook = trn_boot._ntff_profile_via_ctypes("/opt/axon/libaxon_pjrt.so")
        except Exception:
            pass
        mod = types.ModuleType("antenv.axon_hooks")
        mod.get_axon_ntff_profile_hook = lambda: hook
        sys.modules["antenv.axon_hooks"] = mod

    orig_upload = bass_utils.upload_artifacts
    if not getattr(orig_upload, "_safe", False):

        def _safe_upload(tmpdir):
            try:
                return orig_upload(tmpdir)
            except Exception:
                return tmpdir

        _safe_upload._safe = True
        bass_utils.upload_artifacts = _safe_upload


_ensure_trace_support()

N_CORES = 8
B, PH, PW, C = 4, 128, 128, 16       # parent_x shape
GROUPS = 128 // C                    # 8 channel-groups per partition dim
PIX_PER_CORE = B * PH * PW // N_CORES  # 8192
NFREE = PIX_PER_CORE // GROUPS       # 1024 pixels per group
HALF = NFREE // 2
F32 = mybir.dt.float32
F16 = mybir.dt.float16
BF16 = mybir.dt.bfloat16


def build_nc(npre: int = 14) -> bass.Bass:
    """Per-core pipeline (only SP/ACT own HW DGE queues; per-queue DMA
    throughput is PACKET-rate bound at ~98ns/packet/engine, so x and y
    move as single whole-tensor DMAs with 2KB rows = max packet size):
      sync  (SP) : ONE DMA bringing the fp16 conv matrix + all of x
                   (W packed as the first 128 columns, keeping 2.25KB
                   contiguous rows = max packet size); then the first
                   y half out (y packets drain under walrus's fixed ~6us
                   semaphore-reset epilogue, off the clock)
      scalar(ACT): DMA b (f32 bias col), then bias add + fp16 convert of
                   the second half (ACT reads PSUM; GPSIMD cannot)
      tensor(PE) : npre junk matmuls (hold the HAM clock gate open while
                   x is in flight), then 2 real fp16 matmuls
      vector(DVE): bias add + PSUM f32 -> SBUF fp16 convert, first half
                   (runs in parallel with ACT's second half)
    """
    nc = bass.Bass()
    x_ext = nc.declare_dram_parameter("x", [128, 128 + NFREE], F16, isOutput=False)
    b_ext = nc.declare_dram_parameter("b", [128, 1], F32, isOutput=False)
    y_ext = nc.declare_dram_parameter("y", [128, NFREE], F16, isOutput=True)

    with (
        nc.sbuf_tensor("x_sb", [128, 128 + NFREE], F16) as x_sb,
        nc.sbuf_tensor("b_sb", [128, 1], F32) as b_sb,
        nc.sbuf_tensor("y_sb", [128, NFREE], F16) as y_sb,
        nc.sbuf_tensor("junk_sb", [128, 256], BF16) as junk_sb,
        nc.sbuf_tensor("scr_sb", [128, 1], F16) as scr_sb,
        nc.psum_tensor("ps0", [128, HALF], F32) as ps0,
        nc.psum_tensor("ps1", [128, HALF], F32) as ps1,
        nc.psum_tensor("ps_junk", [128, 256], F32) as ps_junk,
        nc.Block(no_gpsimd_drain=True) as block,
        nc.semaphore("bsem") as bsem,
        nc.semaphore("dsem") as dsem,
        nc.semaphore("msem") as msem,
        nc.semaphore("a0sem") as a0sem,
        nc.semaphore("a1sem") as a1sem,
        nc.semaphore("osem") as osem,
    ):

        @block.sync
        def _(sync):
            sync.dma_start(out=x_sb[:], in_=x_ext[:]).then_inc(dsem, 16)
            sync.wait_ge(a0sem, 1)
            sync.dma_start(out=y_ext[:, 0:HALF], in_=y_sb[:, 0:HALF]).then_inc(
                osem, 16
            )
            # no completion wait: the framework's end-of-program DRAIN
            # already waits out the DGE queue

        @block.scalar
        def _(scalar):
            scalar.dma_start(out=b_sb[:], in_=b_ext[:]).then_inc(bsem, 16)
            # dummy 1-col activation: absorbs the lazy act-table load
            # (~2us on first ACT activation) while the x DMA is in flight
            scalar.add(scr_sb[:], junk_sb[:, 0:1], 0.0)
            scalar.wait_ge(bsem, 16)
            scalar.wait_ge(msem, 2)
            scalar.add(y_sb[:, HALF:NFREE], ps1[:], b_sb[:]).then_inc(a1sem, 1)
            scalar.wait_ge(a1sem, 1)
            scalar.dma_start(
                out=y_ext[:, HALF:NFREE], in_=y_sb[:, HALF:NFREE]
            ).then_inc(osem, 16)

        @block.tensor
        def _(tensor):
            # Junk matmuls read junk_sb UNINITIALIZED: values are irrelevant
            # (ps_junk is never read) and skipping a memset dependency lets
            # the PE busy-window start at engine release. (CoreSim would
            # reject the uninitialized read; hardware doesn't care.)
            for _ in range(npre):
                tensor.matmul(
                    ps_junk[:], junk_sb[:, 0:128], junk_sb[:],
                    start=True, stop=True, skip_group_check=True,
                )
            tensor.wait_ge(dsem, 16)
            tensor.matmul(
                ps0[:], x_sb[:, 0:128], x_sb[:, 128 : 128 + HALF],
                start=True, stop=True,
            ).then_inc(msem, 1)
            tensor.matmul(
                ps1[:], x_sb[:, 0:128], x_sb[:, 128 + HALF : 128 + NFREE],
                start=True, stop=True,
            ).then_inc(msem, 1)

        @block.vector
        def _(vector):
            vector.wait_ge(bsem, 16)
            vector.wait_ge(msem, 1)
            vector.tensor_scalar_add(y_sb[:, 0:HALF], ps0[:], b_sb[:]).then_inc(
                a0sem, 1
            )

    return nc


_NC = None


def _get_nc() -> bass.Bass:
    global _NC
    if _NC is None:
        _NC = build_nc()
    return _NC


def _pack_inputs(parent_x, conv_kernel, conv_bias, Wv):
    wv = float(np.asarray(Wv).reshape(-1)[0])
    W = (np.asarray(conv_kernel, np.float32) * wv).astype(np.float16)    # (16,16)
    bias = (np.asarray(conv_bias, np.float32) * wv).astype(np.float32)   # (16,)

    # stationary: out = S.T @ rhs with S[16a+c, 16a+f] = W[c, f]
    WD = np.zeros((128, 128), np.float16)
    BD = np.zeros((128, 1), np.float32)
    for a in range(GROUPS):
        WD[C * a : C * (a + 1), C * a : C * (a + 1)] = W
        BD[C * a : C * (a + 1), 0] = bias
    # x packed per core: row 16a+c = channel c of pixel-group a; the
    # stationary matrix rides along as the first 128 columns of each shard
    xf = np.ascontiguousarray(parent_x, dtype=np.float32).reshape(
        N_CORES, GROUPS, NFREE, C
    )
    xp = np.ascontiguousarray(xf.transpose(0, 1, 3, 2)).reshape(N_CORES, 128, NFREE)
    wx = np.concatenate(
        [np.broadcast_to(WD, (N_CORES, 128, 128)), xp.astype(np.float16)], axis=2
    )
    return np.ascontiguousarray(wx), BD


def _unpack_output(y_shards):
    # y_shards: (8, 128, 1024) fp16 with row 16a+f = channel f of group a
    y = np.asarray(y_shards).astype(np.float32).reshape(N_CORES, GROUPS, C, NFREE)
    y = y.transpose(0, 1, 3, 2).reshape(B, PH, PW, C)
    out = np.broadcast_to(
        y[:, :, None, :, None, :], (B, PH, 2, PW, 2, C)
    ).reshape(B, 2 * PH, 2 * PW, C)
    return np.ascontiguousarray(out)


def _make_in_maps(inputs):
    wx, BD = _pack_inputs(
        inputs["parent_x"], inputs["conv_kernel"], inputs["conv_bias"], inputs["Wv"]
    )
    return [{"x": wx[k], "b": BD} for k in range(N_CORES)]


def kernel(parent_x, leaf_x, conv_kernel, conv_bias, Wq, Wk, Wv, **_unused):
    wx, BD = _pack_inputs(parent_x, conv_kernel, conv_bias, Wv)
    in_maps = [{"x": wx[k], "b": BD} for k in range(N_CORES)]
    nc = _get_nc()
    res = run_bass_kernel_spmd(nc, in_maps, list(range(N_CORES))).results
    y = np.stack([res[k]["y"] for k in range(N_CORES)])
    return _unpack_output(y)


if __name__ == "__main__":
    rng = np.random.default_rng(0)
    inputs = {
        "parent_x": rng.standard_normal((B, PH, PW, C)).astype(np.float32),
        "leaf_x": rng.standard_normal((B, 2 * PH, 2 * PW, C)).astype(np.float32),
        "conv_kernel": (rng.standard_normal((C, C)) * 0.1).astype(np.float32),
        "conv_bias": (rng.standard_normal(C) * 0.1).astype(np.float32),
        "Wq": rng.standard_normal((1, C)).astype(np.float32),
        "Wk": rng.standard_normal((1, C)).astype(np.float32),
        "Wv": rng.standard_normal((1, 1)).astype(np.float32),
    }
    out = kernel(**inputs)
    wv = float(inputs["Wv"][0, 0])
    s = inputs["parent_x"] @ (inputs["conv_kernel"] * wv) + inputs["conv_bias"] * wv
    exp = np.repeat(np.repeat(s, 2, axis=1), 2, axis=2)
    rel = np.linalg.norm(out - exp) / np.linalg.norm(exp)
    print("self-check rel err:", rel)
